# revision 65
# baseline (speedup 1.0000x reference)
"""Multi-head attention + output projection on 8 Trainium2 NeuronCores.

Problem (hardcoded): B=4, N=M=2048, D_IN=D_OUT=512, H=8, HD=VD=64.
  out = softmax(q @ k^T / sqrt(64)) @ v, heads concat, @ W_out.T + b_out

Sharding: each core owns (batch b = core//2, query-row half = core%2):
  q-chunk [1024, 512], full k/v for that batch, full W_out. All 8 heads are
  computed locally, so no collectives are needed; the host concatenates the
  8 disjoint [1024, 512] output chunks.

Device algorithm per core — engine-balanced around four ideas:
  1. exp split across ACT and DVE with per-head S tiles: each unit
     (pair, jt, i-half) produces two [128, 512] S psum tiles, one per
     head. ACT exps head0 natively (bf16 out); DVE exps head1 with a
     Schraudolph bit-trick — one tensor_scalar i16(S*c1+c2) whose int16
     result IS the bf16 bit pattern of 2^(0.125*S*log2e) (~1.8% RMS on
     those columns; the softmax ratio cancels the mean part, ~1.3%
     end-to-end vs the 2e-2 gate). Every 17th unit ACT takes head1 too,
     keeping both engines at the PE's pace. Each psum tile has exactly
     ONE reader engine: the tile framework serializes cross-engine PSUM
     readers, and GPSIMD cannot touch PSUM at all (so Pool sits out).
  2. PV uses P^T as the *stationary* matmul operand (lhsT) and the 64-col
     v tile as the moving one, so each accumulation step bills only 64
     columns instead of 1024: O[i,d] psum accumulates per 128-i-slice
     over the 16 j-tiles; sumexp rides as 1-col ones matmuls. PSUM
     accumulation groups are per 2KB bank: one start/stop per o-bank and
     sum-bank per pair (start lazily zero-marks the bank; each byte is
     claimed once). Normalization = DVE recip + broadcast multiply; a PE
     transpose chain (8x [128,128] bf16, one group) writes O^T into the
     dead sum bank, freeing a PSUM bank that gives head1's S tiles
     3-deep buffering.
  3. bf16 everywhere off the f32 PSUM accumulators (q/k/v/P/W_out),
     halving DMA and enabling the small-free-dim matmuls at 1 cyc/col.
     b_out is added on the host (it is zeros here anyway).
  4. Software-pipelined emission: PV lags QK/exp by LAG=4 units so the
     in-order PE SEQ never parks on an exp-produced weight; pair-boundary
     transposes are deferred one unit. The last pair closes its slice 0-3
     accumulation groups one unit early (ih1 writes run with the group
     check skipped, exact via the pending-zero marks), so normalize /
     transpose / projection of the first i-half and the pair-0..2 partial
     projections of the second overlap the final unit's compute.
  Output is bf16 (host converts to f32 and adds b_out); the eight output
  DMAs alternate between the SP and Pool (SWDGE) queues to halve the
  serial issue cost on the tail.
  Cost model: ~105.2us/core (PE 91us busy: QK 131072 cyc + PV 66560 +
  transposes + 16384 proj; ACT ~88us; DVE ~90us), rel err ~1.3e-2.
  Remaining idle is latency-bound: ~3us first-DMA fixed costs (SEQ/DGE/
  sem-prop), ~3.5us s-buffer WAR chain jitter at ~0.08us/2-unit margin,
  ~5us closing chain (last PV -> norm -> transpose -> proj -> copy ->
  DMA -> drain).
"""

import numpy as np

B, N, M, D, H, HD = 4, 2048, 2048, 512, 8, 64
NLOC = N // 2  # query rows per core
NCORES = 8
JT = M // 128  # 16 j-tiles
NU = 2  # i-halves per jt (512 cols each)
LOG2E = 1.4426950408889634
C_SCH = 0.0573  # Schraudolph bias minimizing RMS rel err of the bf16 bit-exp
C1 = float(np.float32(0.125 * LOG2E * 128.0))
C2 = float(np.float32((127.0 - C_SCH) * 128.0 + 0.5))  # +0.5: i16 cast truncates


def _build_bass(debug=False):
    import concourse.mybir as mybir
    import concourse.tile as tile
    from concourse import bacc

    f32 = mybir.dt.float32
    bf = mybir.dt.bfloat16
    i16 = mybir.dt.int16
    Exp = mybir.ActivationFunctionType.Exp
    mult = mybir.AluOpType.mult
    add = mybir.AluOpType.add

    nc = bacc.Bacc()
    qt_d = nc.dram_tensor("qt", [4, 128, NLOC], bf, kind="ExternalInput")
    kt_d = nc.dram_tensor("kt", [4, 128, M], bf, kind="ExternalInput")
    va_d = nc.dram_tensor("va", [128, JT, H, HD], bf, kind="ExternalInput")
    wt_d = nc.dram_tensor("wt", [4, 128, D], bf, kind="ExternalInput")
    idn_d = nc.dram_tensor("idn", [128, 128], bf, kind="ExternalInput")
    out_d = nc.dram_tensor("out", [NLOC, D], bf, kind="ExternalOutput")
    if debug:
        dbg = {
            "dbg_p": nc.dram_tensor("dbg_p", [128, 2, 512], f32, kind="ExternalOutput"),
            "dbg_o": nc.dram_tensor("dbg_o", [128, 8, HD], f32, kind="ExternalOutput"),
            "dbg_sm": nc.dram_tensor("dbg_sm", [128, 16], f32, kind="ExternalOutput"),
            "dbg_on": nc.dram_tensor("dbg_on", [128, 8, 128], f32, kind="ExternalOutput"),
            "dbg_ot": nc.dram_tensor("dbg_ot", [128, NLOC], f32, kind="ExternalOutput"),
        }

    with tile.TileContext(nc) as tc:
        with (
            tc.tile_pool(name="persist", bufs=1) as persist,
            tc.tile_pool(name="pt", bufs=12) as ptp,
            tc.tile_pool(name="work", bufs=2) as work,
            tc.tile_pool(name="ps_s", bufs=1, space="PSUM") as ps_s,
            tc.tile_pool(name="ps_o", bufs=1, space="PSUM") as ps_o,
        ):
            kt_sb = [persist.tile([128, M], bf, tag=f"kt{o}", name=f"kt{o}") for o in range(4)]
            qt_sb = [persist.tile([128, NLOC], bf, tag=f"qt{o}", name=f"qt{o}") for o in range(4)]
            va_sb = persist.tile([128, JT, H, HD], bf)
            wt_sb = persist.tile([128, 4, D], bf)
            idn_sb = persist.tile([128, 128], bf)
            ones_c = persist.tile([128, 1], bf)
            ot_sb = [persist.tile([128, NLOC], bf, tag=f"ot{o}", name=f"ot{o}") for o in range(4)]

            nc.vector.memset(ones_c, 1.0)
            # PE pstate warmup: ~3us of dummy matmuls into the (not yet
            # used) sum bank so the clock ramp finishes before the first
            # real QK; the first pair's sum generation waits on the WAW and
            # starts well after these complete.
            dumw = persist.tile([128, 512], bf)
            nc.vector.memset(dumw, 1.0)
            dum_ps = ps_o.tile([128, 512], f32, tag="sum", name="dum_ps")
            for _ in range(0):
                nc.tensor.matmul(
                    dum_ps, lhsT=dumw[:, 0:128], rhs=dumw, start=True, stop=True
                )

            # DMA schedule ordered by first use: first QK needs kt0 jt0 +
            # qt0 half0; first PV needs va jt0; then stream the rest.
            nc.sync.dma_start(kt_sb[0][:, 0:128], kt_d[0, :, 0:128])
            nc.scalar.dma_start(qt_sb[0][:, 0:512], qt_d[0, :, 0:512])
            nc.sync.dma_start(qt_sb[0][:, 512:1024], qt_d[0, :, 512:1024])
            nc.gpsimd.dma_start(kt_sb[0][:, 128:512], kt_d[0, :, 128:512])
            nc.scalar.dma_start(va_sb[:, 0:1], va_d[:, 0:1])
            nc.sync.dma_start(va_sb[:, 1:4], va_d[:, 1:4])
            nc.sync.dma_start(kt_sb[0][:, 512:2048], kt_d[0, :, 512:2048])
            nc.sync.dma_start(va_sb[:, 4:10], va_d[:, 4:10])
            nc.sync.dma_start(va_sb[:, 10:16], va_d[:, 10:16])
            for o in range(1, 4):
                nc.sync.dma_start(kt_sb[o], kt_d[o])
                nc.sync.dma_start(qt_sb[o], qt_d[o])
            nc.sync.dma_start(idn_sb, idn_d[:])
            for o in range(4):
                nc.sync.dma_start(wt_sb[:, o : o + 1], wt_d[o : o + 1])

            # per-pair psum/staging tiles, captured per pair generation so
            # deferred work (normalize/transpose) reads the right tiles
            pair_state = {}

            def emit_qk_exp(hp, jt, ih, u):
                # One PSUM tile per head per unit, each with exactly ONE
                # reader engine: the tile framework serializes cross-engine
                # READERS of a PSUM tile (reader chaining), so sharing one S
                # tile between ACT and DVE would run them back-to-back.
                buf = u % 2
                s0 = ps_s.tile([128, 512], f32, tag=f"sa{buf}", name="s0")
                s1 = ps_s.tile([128, 512], f32, tag=f"sb{u % 3}", name="s1")
                for h01, st in ((0, s0), (1, s1)):
                    nc.tensor.matmul(
                        st,
                        lhsT=kt_sb[hp][64 * h01 : 64 * h01 + 64, jt * 128 : (jt + 1) * 128],
                        rhs=qt_sb[hp][64 * h01 : 64 * h01 + 64, ih * 512 : (ih + 1) * 512],
                        start=True,
                        stop=True,
                    )
                # GPSIMD cannot read PSUM on real HW, so only ACT and DVE
                # can consume S. DVE does head1's bit-trick exp on most
                # units; every 9th unit ACT takes head1 too (native exp) to
                # keep the DVE total under the PE-work bound.
                pa = ptp.tile([128, 512], bf, tag=f"pa{buf}", name="pa")
                nc.scalar.activation(pa, s0, Exp, scale=0.125)
                px = ptp.tile([128, 512], bf, tag=f"px{buf}", name="px")
                if u % 17 == 5:
                    # ACT absorbs head1 too on this cadence to balance DVE
                    nc.scalar.activation(px, s1, Exp, scale=0.125)
                else:
                    nc.vector.tensor_scalar(
                        px[:].bitcast(i16), s1, C1, C2, mult, add
                    )
                return (pa, px)

            def emit_pv(hp, jt, ih, p):
                o_tiles, sum_ps = pair_state[hp]["o"], pair_state[hp]["sum"]
                pa, px = p
                slices = [
                    [pa[:, 0:128], pa[:, 128:256], pa[:, 256:384], pa[:, 384:512]],
                    [px[:, 0:128], px[:, 128:256], px[:, 256:384], px[:, 384:512]],
                ]
                # PSUM accumulation groups are per 2KB zero-region (bank):
                # exactly one start (first touch zero-marks the whole bank)
                # and one stop (last touch) per o-bank / sum-bank per pair.
                # Last pair: slices 0-3 form their own accumulation group
                # that closes at (jt15, ih0), so normalize/transpose/proj of
                # the first i-half overlaps the final i-half's compute. The
                # ih1 writes keep accumulating with the group check skipped
                # (their bytes still carry the pending-zero marks from the
                # group-A start, so values stay exact).
                last = hp == 3
                for h01 in range(2):
                    for sl in range(4):
                        gsl = ih * 4 + sl
                        lhsT = slices[h01][sl]
                        o_stop = (
                            (jt == JT - 1 and gsl in (3, 7))
                            if last
                            else (jt == JT - 1 and gsl == 7)
                        )
                        nc.tensor.matmul(
                            o_tiles[h01][:, gsl, :],
                            lhsT=lhsT,
                            rhs=va_sb[:, jt, 2 * hp + h01, :],
                            start=(jt == 0 and gsl == 0),
                            stop=o_stop,
                            skip_group_check=(last and ih == 1),
                        )
                        s_stop = (
                            (jt == JT - 1 and gsl in (3, 7) and h01 == 1)
                            if last
                            else (jt == JT - 1 and gsl == 7 and h01 == 1)
                        )
                        nc.tensor.matmul(
                            sum_ps[:, h01 * 8 + gsl : h01 * 8 + gsl + 1],
                            lhsT=lhsT,
                            rhs=ones_c,
                            start=(jt == 0 and gsl == 0 and h01 == 0),
                            stop=s_stop,
                            skip_group_check=(last and ih == 1),
                        )

            def emit_norm(hp):
                # softmax normalization for the whole pair (PSUM accumulation
                # groups close at the pair's last PV, and mid-group reads are
                # not allowed): recip of sumexp, broadcast-multiply, bf16 out
                # staged for transpose
                st = pair_state[hp]
                o_tiles, sum_ps = st["o"], st["sum"]
                rc, on = st["rc"], st["on"]
                nc.vector.reciprocal(rc, sum_ps[:, 0:16])
                for h01 in range(2):
                    nc.vector.tensor_tensor(
                        on[:, :, 64 * h01 : 64 * h01 + 64],
                        o_tiles[h01],
                        rc[:, h01 * 8 : h01 * 8 + 8].unsqueeze(2).broadcast_to(
                            [128, 8, HD]
                        ),
                        mult,
                    )
                if debug and hp == 0:
                    dbg_o = work.tile([128, 8, HD], f32, tag="dbg_o", name="dbg_o")
                    nc.vector.tensor_copy(dbg_o, o_tiles[0])
                    nc.sync.dma_start(dbg["dbg_o"][:], dbg_o)

            def emit_norm_half(ih):
                st = pair_state[3]
                o_tiles, sum_ps = st["o"], st["sum"]
                rc, on = st["rc"], st["on"]
                a = ih * 4
                for h01 in range(2):
                    nc.vector.reciprocal(
                        rc[:, h01 * 8 + a : h01 * 8 + a + 4],
                        sum_ps[:, h01 * 8 + a : h01 * 8 + a + 4],
                    )
                    nc.vector.tensor_tensor(
                        on[:, a : a + 4, 64 * h01 : 64 * h01 + 64],
                        o_tiles[h01][:, a : a + 4, :],
                        rc[:, h01 * 8 + a : h01 * 8 + a + 4]
                        .unsqueeze(2)
                        .broadcast_to([128, 4, HD]),
                        mult,
                    )

            proj_state = {}

            def emit_proj_partial(c, tag):
                # pairs 0-2 of chunk c (group left open; finished later once
                # ot3 lands). Banks: the sa/sb psum tags free up as the last
                # units retire, giving four chunks in flight.
                csl = slice(c * 128, (c + 1) * 128)
                ps_f = ps_s.tile([128, D], f32, tag=tag, name="ps_f")
                proj_state[c] = ps_f
                for o in range(3):
                    nc.tensor.matmul(
                        ps_f,
                        lhsT=ot_sb[o][:, csl],
                        rhs=wt_sb[:, o, :],
                        start=(o == 0),
                        stop=False,
                    )

            def emit_proj_finish(c):
                csl = slice(c * 128, (c + 1) * 128)
                ps_f = proj_state.pop(c)
                nc.tensor.matmul(
                    ps_f, lhsT=ot_sb[3][:, csl], rhs=wt_sb[:, 3, :],
                    start=False, stop=True,
                )
                f_sb = ptp.tile([128, D], bf, tag="fin", name="f_sb")
                if c % 2 == 0:
                    nc.vector.tensor_copy(f_sb, ps_f)
                else:
                    nc.scalar.copy(f_sb, ps_f)
                # alternate output-DMA queues: the SP SEQ costs ~0.65us per
                # issue and head-of-line blocks on the copy, so the idle
                # Pool (SWDGE) queue takes every other chunk
                if c % 2 == 0:
                    nc.sync.dma_start(out_d[csl, :], f_sb)
                else:
                    nc.gpsimd.dma_start(out_d[csl, :], f_sb)

            def emit_proj(c, tag):
                emit_proj_partial(c, tag)
                emit_proj_finish(c)

            def emit_tail_half(ih):
                # transposes of the half's 4 slices, copy to SBUF, then the
                # matching projection chunks
                st = pair_state[3]
                on = st["on"]
                if ih == 0:
                    tpl = ps_s.tile([128, 512], f32, tag="sb0", name="tp_last")
                    st["tpl"] = tpl
                    tgt = tpl[:].bitcast(bf)
                else:
                    tgt = st["sum"][:].bitcast(bf)
                for sl in range(ih * 4, ih * 4 + 4):
                    nc.tensor.matmul(
                        tgt[:, (sl % 4) * 128 : (sl % 4) * 128 + 128]
                        if ih == 0
                        else tgt[:, sl * 128 : sl * 128 + 128],
                        lhsT=on[:, sl, :],
                        rhs=idn_sb,
                        is_transpose=True,
                        start=(sl % 4 == 0),
                        stop=(sl % 4 == 3),
                    )
                src_cols = slice(0, 512) if ih == 0 else slice(512, 1024)
                if ih == 0:
                    nc.vector.tensor_copy(
                        ot_sb[3][:, 0:512], tgt[:, src_cols]
                    )
                    for c, tag in ((0, "sa0"), (1, "sa1"), (2, "sb2"), (3, "sa0")):
                        emit_proj(c, tag)
                    for c, tag in ((4, "sb1"), (5, "sa1"), (6, "sb2"), (7, "sa0")):
                        emit_proj_partial(c, tag)
                else:
                    nc.scalar.copy(ot_sb[3][:, 512:1024], tgt[:, src_cols])
                    for c in range(4, 8):
                        emit_proj_finish(c)

            def emit_transpose(hp):
                # The sum bank is dead after the normalize reads it, so the
                # pair's 8 transposes reuse it (one accumulation group, each
                # byte written exactly once), then a DMA moves O^T to SBUF —
                # no engine cycles spent on the copy.
                st = pair_state[hp]
                on = st["on"]
                tp = st["sum"][:].bitcast(bf)
                for sl in range(8):
                    nc.tensor.matmul(
                        tp[:, sl * 128 : (sl + 1) * 128],
                        lhsT=on[:, sl, :],
                        rhs=idn_sb,
                        is_transpose=True,
                        start=(sl == 0),
                        stop=(sl == 7),
                    )
                if hp == 3:
                    nc.scalar.copy(ot_sb[hp], tp)
                else:
                    nc.vector.tensor_copy(ot_sb[hp], tp)

            def alloc_pair(hp):
                pair_state[hp] = {
                    "o": [
                        ps_o.tile([128, 8, HD], f32, tag=f"o{h01}", name=f"o{h01}")
                        for h01 in range(2)
                    ],
                    "sum": ps_o.tile([128, 512], f32, tag="sum", name="sum"),
                    "rc": work.tile([128, 16], f32, tag="rc", name="rc"),
                    "on": work.tile([128, 8, 128], bf, tag="on", name="on"),
                }
                if hp - 2 in pair_state:
                    del pair_state[hp - 2]

            # Software-pipelined emission, PV lagging QK/exp by LAG units so
            # the PE never waits on the exp engines in steady state. Pair
            # boundary work is interleaved: norm right after the pair's last
            # PV, transposes one iteration later (so the PE meets them after
            # the DVE normalize has finished), all before the next pair
            # reuses the same psum tags.
            LAG = 5
            units = [(hp, jt, ih) for hp in range(4) for jt in range(JT) for ih in range(NU)]
            transposes = []  # (due_iter, hp)
            tails = []  # (due_iter, ih) for the last pair

            def emit_iter(u):
                # PV of u-LAG first: its inputs are ready, so the in-order
                # PE SEQ does useful work while QK(u) waits out the s-buffer
                # WAR on the exp engines of u-2.
                pu = u - LAG
                if 0 <= pu < len(units):
                    php, pjt, pih = units[pu]
                    if pjt == 0 and pih == 0:
                        alloc_pair(php)
                    emit_pv(php, pjt, pih, pending_p[pu])
                    pending_p[pu] = None
                # norm before this unit's exp: it then sits ahead of the
                # DVE's next bit-exp in the queue, starting the pair-close
                # chain one exp earlier (sb's 3-deep buffering absorbs the
                # delayed exp)
                if 0 <= pu < len(units):
                    php, pjt, pih = units[pu]
                    if pjt == JT - 1:
                        if php == 3:
                            emit_norm_half(pih)
                            tails.append((u + 1, pih))
                        elif pih == 1:
                            emit_norm(php)
                            transposes.append((u, php))
                if u < len(units):
                    hp, jt, ih = units[u]
                    emit_qk_exp(hp, jt, ih, u)
                while transposes and transposes[0][0] <= u:
                    _, thp = transposes.pop(0)
                    emit_transpose(thp)
                while tails and tails[0][0] <= u:
                    _, tih = tails.pop(0)
                    emit_tail_half(tih)

            pending_p = {}
            orig_qk = emit_qk_exp

            def emit_qk_exp_wrap(hp, jt, ih, u):
                pending_p[u] = orig_qk(hp, jt, ih, u)

            emit_qk_exp = emit_qk_exp_wrap
            for u in range(len(units) + LAG):
                emit_iter(u)
            while transposes:
                _, thp = transposes.pop(0)
                emit_transpose(thp)
            while tails:
                _, tih = tails.pop(0)
                emit_tail_half(tih)


    nc.finalize()
    return nc


def _host_prep(q, k, v, W_out, b_out):
    """Shard + lay out inputs per core (pure layout: transpose/pack/bf16)."""
    import ml_dtypes

    bf16 = ml_dtypes.bfloat16
    q = np.asarray(q, dtype=np.float32)
    k = np.asarray(k, dtype=np.float32)
    v = np.asarray(v, dtype=np.float32)
    W_out = np.asarray(W_out, dtype=np.float32)
    b_out = np.asarray(b_out, dtype=np.float32)

    qT = np.ascontiguousarray(q.transpose(0, 2, 1)).astype(bf16)  # [B, D, N]
    kT = np.ascontiguousarray(k.transpose(0, 2, 1)).astype(bf16)  # [B, D, M]
    # va[p, jt, h, hd] = v[b, jt*128 + p, h*64 + hd]
    va = np.ascontiguousarray(
        v.reshape(B, JT, 128, H, HD).transpose(0, 2, 1, 3, 4)
    ).astype(bf16)
    wt = np.ascontiguousarray(W_out.T.reshape(4, 128, D)).astype(bf16)
    idn = np.eye(128, dtype=np.float32).astype(bf16)

    in_maps = []
    for c in range(NCORES):
        b_, ihalf = divmod(c, 2)
        in_maps.append(
            {
                "qt": np.ascontiguousarray(
                    qT[b_, :, ihalf * NLOC : (ihalf + 1) * NLOC].reshape(4, 128, NLOC)
                ),
                "kt": np.ascontiguousarray(kT[b_].reshape(4, 128, M)),
                "va": va[b_],
                "wt": wt,
                "idn": idn,
            }
        )
    return in_maps


def kernel(q, k, v, W_out, b_out):
    from concourse.bass_utils import run_bass_kernel_spmd

    nc = _build_bass()
    in_maps = _host_prep(q, k, v, W_out, b_out)
    res = run_bass_kernel_spmd(nc, in_maps, core_ids=list(range(NCORES)))
    out = np.empty((B, N, D), dtype=np.float32)
    for c, r_ in enumerate(res.results):
        b_, ihalf = divmod(c, 2)
        out[b_, ihalf * NLOC : (ihalf + 1) * NLOC, :] = r_["out"].astype(
            np.float32
        )
    b_vec = np.asarray(b_out, dtype=np.float32)
    if b_vec.any():
        out += b_vec[None, None, :]
    return out


# revision 67
# speedup vs baseline: 1.0008x; 1.0008x over previous
"""Multi-head attention + output projection on 8 Trainium2 NeuronCores.

Problem (hardcoded): B=4, N=M=2048, D_IN=D_OUT=512, H=8, HD=VD=64.
  out = softmax(q @ k^T / sqrt(64)) @ v, heads concat, @ W_out.T + b_out

Sharding: each core owns (batch b = core//2, query-row half = core%2):
  q-chunk [1024, 512], full k/v for that batch, full W_out. All 8 heads are
  computed locally, so no collectives are needed; the host concatenates the
  8 disjoint [1024, 512] output chunks.

Device algorithm per core — engine-balanced around four ideas:
  1. exp split across ACT and DVE with per-head S tiles: each unit
     (pair, jt, i-half) produces two [128, 512] S psum tiles, one per
     head. ACT exps head0 natively (bf16 out); DVE exps head1 with a
     Schraudolph bit-trick — one tensor_scalar i16(S*c1+c2) whose int16
     result IS the bf16 bit pattern of 2^(0.125*S*log2e) (~1.8% RMS on
     those columns; the softmax ratio cancels the mean part, ~1.3%
     end-to-end vs the 2e-2 gate). Every 17th unit ACT takes head1 too,
     keeping both engines at the PE's pace. Each psum tile has exactly
     ONE reader engine: the tile framework serializes cross-engine PSUM
     readers, and GPSIMD cannot touch PSUM at all (so Pool sits out).
  2. PV uses P^T as the *stationary* matmul operand (lhsT) and the 64-col
     v tile as the moving one, so each accumulation step bills only 64
     columns instead of 1024: O[i,d] psum accumulates per 128-i-slice
     over the 16 j-tiles; sumexp rides as 1-col ones matmuls. PSUM
     accumulation groups are per 2KB bank: one start/stop per o-bank and
     sum-bank per pair (start lazily zero-marks the bank; each byte is
     claimed once). Normalization = DVE recip + broadcast multiply; a PE
     transpose chain (8x [128,128] bf16, one group) writes O^T into the
     dead sum bank, freeing a PSUM bank that gives head1's S tiles
     3-deep buffering.
  3. bf16 everywhere off the f32 PSUM accumulators (q/k/v/P/W_out),
     halving DMA and enabling the small-free-dim matmuls at 1 cyc/col.
     b_out is added on the host (it is zeros here anyway).
  4. Software-pipelined emission: PV lags QK/exp by LAG=4 units so the
     in-order PE SEQ never parks on an exp-produced weight; pair-boundary
     transposes are deferred one unit. The last pair closes its slice 0-3
     accumulation groups one unit early (ih1 writes run with the group
     check skipped, exact via the pending-zero marks), so normalize /
     transpose / projection of the first i-half and the pair-0..2 partial
     projections of the second overlap the final unit's compute.
  Output is bf16 (host converts to f32 and adds b_out); the eight output
  DMAs alternate between the SP and Pool (SWDGE) queues to halve the
  serial issue cost on the tail.
  Cost model: ~105.2us/core (PE 91us busy: QK 131072 cyc + PV 66560 +
  transposes + 16384 proj; ACT ~88us; DVE ~90us), rel err ~1.3e-2.
  Remaining idle is latency-bound: ~3us first-DMA fixed costs (SEQ/DGE/
  sem-prop), ~3.5us s-buffer WAR chain jitter at ~0.08us/2-unit margin,
  ~5us closing chain (last PV -> norm -> transpose -> proj -> copy ->
  DMA -> drain).
"""

import numpy as np

B, N, M, D, H, HD = 4, 2048, 2048, 512, 8, 64
NLOC = N // 2  # query rows per core
NCORES = 8
JT = M // 128  # 16 j-tiles
NU = 2  # i-halves per jt (512 cols each)
LOG2E = 1.4426950408889634
C_SCH = 0.0573  # Schraudolph bias minimizing RMS rel err of the bf16 bit-exp
C1 = float(np.float32(0.125 * LOG2E * 128.0))
C2 = float(np.float32((127.0 - C_SCH) * 128.0 + 0.5))  # +0.5: i16 cast truncates


def _build_bass(debug=False):
    import concourse.mybir as mybir
    import concourse.tile as tile
    from concourse import bacc

    f32 = mybir.dt.float32
    bf = mybir.dt.bfloat16
    i16 = mybir.dt.int16
    Exp = mybir.ActivationFunctionType.Exp
    mult = mybir.AluOpType.mult
    add = mybir.AluOpType.add

    nc = bacc.Bacc()
    qt_d = nc.dram_tensor("qt", [4, 128, NLOC], bf, kind="ExternalInput")
    kt_d = nc.dram_tensor("kt", [4, 128, M], bf, kind="ExternalInput")
    va_d = nc.dram_tensor("va", [128, JT, H, HD], bf, kind="ExternalInput")
    wt_d = nc.dram_tensor("wt", [4, 128, D], bf, kind="ExternalInput")
    idn_d = nc.dram_tensor("idn", [128, 128], bf, kind="ExternalInput")
    out_d = nc.dram_tensor("out", [NLOC, D], bf, kind="ExternalOutput")
    if debug:
        dbg = {
            "dbg_p": nc.dram_tensor("dbg_p", [128, 2, 512], f32, kind="ExternalOutput"),
            "dbg_o": nc.dram_tensor("dbg_o", [128, 8, HD], f32, kind="ExternalOutput"),
            "dbg_sm": nc.dram_tensor("dbg_sm", [128, 16], f32, kind="ExternalOutput"),
            "dbg_on": nc.dram_tensor("dbg_on", [128, 8, 128], f32, kind="ExternalOutput"),
            "dbg_ot": nc.dram_tensor("dbg_ot", [128, NLOC], f32, kind="ExternalOutput"),
        }

    with tile.TileContext(nc) as tc:
        with (
            tc.tile_pool(name="persist", bufs=1) as persist,
            tc.tile_pool(name="pt", bufs=24) as ptp,
            tc.tile_pool(name="work", bufs=2) as work,
            tc.tile_pool(name="ps_s", bufs=1, space="PSUM") as ps_s,
            tc.tile_pool(name="ps_o", bufs=1, space="PSUM") as ps_o,
        ):
            kt_sb = [persist.tile([128, M], bf, tag=f"kt{o}", name=f"kt{o}") for o in range(4)]
            qt_sb = [persist.tile([128, NLOC], bf, tag=f"qt{o}", name=f"qt{o}") for o in range(4)]
            va_sb = persist.tile([128, JT, H, HD], bf)
            wt_sb = persist.tile([128, 4, D], bf)
            idn_sb = persist.tile([128, 128], bf)
            ones_c = persist.tile([128, 1], bf)
            ot_sb = [persist.tile([128, NLOC], bf, tag=f"ot{o}", name=f"ot{o}") for o in range(4)]

            nc.vector.memset(ones_c, 1.0)
            # PE pstate warmup: ~3us of dummy matmuls into the (not yet
            # used) sum bank so the clock ramp finishes before the first
            # real QK; the first pair's sum generation waits on the WAW and
            # starts well after these complete.
            dumw = persist.tile([128, 512], bf)
            nc.vector.memset(dumw, 1.0)
            dum_ps = ps_o.tile([128, 512], f32, tag="sum", name="dum_ps")
            for _ in range(0):
                nc.tensor.matmul(
                    dum_ps, lhsT=dumw[:, 0:128], rhs=dumw, start=True, stop=True
                )

            # DMA schedule ordered by first use: first QK needs kt0 jt0 +
            # qt0 half0; first PV needs va jt0; then stream the rest.
            nc.sync.dma_start(kt_sb[0][:, 0:128], kt_d[0, :, 0:128])
            nc.scalar.dma_start(qt_sb[0][:, 0:512], qt_d[0, :, 0:512])
            nc.sync.dma_start(qt_sb[0][:, 512:1024], qt_d[0, :, 512:1024])
            nc.gpsimd.dma_start(kt_sb[0][:, 128:512], kt_d[0, :, 128:512])
            nc.scalar.dma_start(va_sb[:, 0:1], va_d[:, 0:1])
            nc.sync.dma_start(va_sb[:, 1:4], va_d[:, 1:4])
            nc.sync.dma_start(kt_sb[0][:, 512:2048], kt_d[0, :, 512:2048])
            nc.sync.dma_start(va_sb[:, 4:10], va_d[:, 4:10])
            nc.sync.dma_start(va_sb[:, 10:16], va_d[:, 10:16])
            for o in range(1, 4):
                nc.sync.dma_start(kt_sb[o], kt_d[o])
                nc.sync.dma_start(qt_sb[o], qt_d[o])
            nc.sync.dma_start(idn_sb, idn_d[:])
            for o in range(4):
                nc.sync.dma_start(wt_sb[:, o : o + 1], wt_d[o : o + 1])

            # per-pair psum/staging tiles, captured per pair generation so
            # deferred work (normalize/transpose) reads the right tiles
            pair_state = {}

            def emit_qk_exp(hp, jt, ih, u):
                # One PSUM tile per head per unit, each with exactly ONE
                # reader engine: the tile framework serializes cross-engine
                # READERS of a PSUM tile (reader chaining), so sharing one S
                # tile between ACT and DVE would run them back-to-back.
                buf = u % 2
                s0 = ps_s.tile([128, 512], f32, tag=f"sa{buf}", name="s0")
                s1 = ps_s.tile([128, 512], f32, tag=f"sb{u % 3}", name="s1")
                for h01, st in ((0, s0), (1, s1)):
                    nc.tensor.matmul(
                        st,
                        lhsT=kt_sb[hp][64 * h01 : 64 * h01 + 64, jt * 128 : (jt + 1) * 128],
                        rhs=qt_sb[hp][64 * h01 : 64 * h01 + 64, ih * 512 : (ih + 1) * 512],
                        start=True,
                        stop=True,
                    )
                # GPSIMD cannot read PSUM on real HW, so only ACT and DVE
                # can consume S. DVE does head1's bit-trick exp on most
                # units; every 9th unit ACT takes head1 too (native exp) to
                # keep the DVE total under the PE-work bound.
                pa = ptp.tile([128, 512], bf, tag=f"pa{buf}", name="pa")
                nc.scalar.activation(pa, s0, Exp, scale=0.125)
                px = ptp.tile([128, 512], bf, tag=f"px{buf}", name="px")
                if u % 17 == 5:
                    # ACT absorbs head1 too on this cadence to balance DVE
                    nc.scalar.activation(px, s1, Exp, scale=0.125)
                else:
                    nc.vector.tensor_scalar(
                        px[:].bitcast(i16), s1, C1, C2, mult, add
                    )
                return (pa, px)

            def emit_pv(hp, jt, ih, p):
                o_tiles, sum_ps = pair_state[hp]["o"], pair_state[hp]["sum"]
                pa, px = p
                slices = [
                    [pa[:, 0:128], pa[:, 128:256], pa[:, 256:384], pa[:, 384:512]],
                    [px[:, 0:128], px[:, 128:256], px[:, 256:384], px[:, 384:512]],
                ]
                # PSUM accumulation groups are per 2KB zero-region (bank):
                # exactly one start (first touch zero-marks the whole bank)
                # and one stop (last touch) per o-bank / sum-bank per pair.
                # Last pair: slices 0-3 form their own accumulation group
                # that closes at (jt15, ih0), so normalize/transpose/proj of
                # the first i-half overlaps the final i-half's compute. The
                # ih1 writes keep accumulating with the group check skipped
                # (their bytes still carry the pending-zero marks from the
                # group-A start, so values stay exact).
                last = hp == 3
                for h01 in range(2):
                    for sl in range(4):
                        gsl = ih * 4 + sl
                        lhsT = slices[h01][sl]
                        o_stop = (
                            (jt == JT - 1 and gsl in (3, 7))
                            if last
                            else (jt == JT - 1 and gsl == 7)
                        )
                        nc.tensor.matmul(
                            o_tiles[h01][:, gsl, :],
                            lhsT=lhsT,
                            rhs=va_sb[:, jt, 2 * hp + h01, :],
                            start=(jt == 0 and gsl == 0),
                            stop=o_stop,
                            skip_group_check=(last and ih == 1),
                        )
                        s_stop = (
                            (jt == JT - 1 and gsl in (3, 7) and h01 == 1)
                            if last
                            else (jt == JT - 1 and gsl == 7 and h01 == 1)
                        )
                        nc.tensor.matmul(
                            sum_ps[:, h01 * 8 + gsl : h01 * 8 + gsl + 1],
                            lhsT=lhsT,
                            rhs=ones_c,
                            start=(jt == 0 and gsl == 0 and h01 == 0),
                            stop=s_stop,
                            skip_group_check=(last and ih == 1),
                        )

            def emit_norm(hp):
                # softmax normalization for the whole pair (PSUM accumulation
                # groups close at the pair's last PV, and mid-group reads are
                # not allowed): recip of sumexp, broadcast-multiply, bf16 out
                # staged for transpose
                st = pair_state[hp]
                o_tiles, sum_ps = st["o"], st["sum"]
                rc, on = st["rc"], st["on"]
                nc.vector.reciprocal(rc, sum_ps[:, 0:16])
                for h01 in range(2):
                    nc.vector.tensor_tensor(
                        on[:, :, 64 * h01 : 64 * h01 + 64],
                        o_tiles[h01],
                        rc[:, h01 * 8 : h01 * 8 + 8].unsqueeze(2).broadcast_to(
                            [128, 8, HD]
                        ),
                        mult,
                    )
                if debug and hp == 0:
                    dbg_o = work.tile([128, 8, HD], f32, tag="dbg_o", name="dbg_o")
                    nc.vector.tensor_copy(dbg_o, o_tiles[0])
                    nc.sync.dma_start(dbg["dbg_o"][:], dbg_o)

            def emit_norm_half(ih):
                st = pair_state[3]
                o_tiles, sum_ps = st["o"], st["sum"]
                rc, on = st["rc"], st["on"]
                a = ih * 4
                for h01 in range(2):
                    nc.vector.reciprocal(
                        rc[:, h01 * 8 + a : h01 * 8 + a + 4],
                        sum_ps[:, h01 * 8 + a : h01 * 8 + a + 4],
                    )
                    nc.vector.tensor_tensor(
                        on[:, a : a + 4, 64 * h01 : 64 * h01 + 64],
                        o_tiles[h01][:, a : a + 4, :],
                        rc[:, h01 * 8 + a : h01 * 8 + a + 4]
                        .unsqueeze(2)
                        .broadcast_to([128, 4, HD]),
                        mult,
                    )

            proj_state = {}

            def emit_proj_partial(c, tag):
                # pairs 0-2 of chunk c (group left open; finished later once
                # ot3 lands). Banks: the sa/sb psum tags free up as the last
                # units retire, giving four chunks in flight.
                csl = slice(c * 128, (c + 1) * 128)
                ps_f = ps_s.tile([128, D], f32, tag=tag, name="ps_f")
                proj_state[c] = ps_f
                for o in range(3):
                    nc.tensor.matmul(
                        ps_f,
                        lhsT=ot_sb[o][:, csl],
                        rhs=wt_sb[:, o, :],
                        start=(o == 0),
                        stop=False,
                    )

            def emit_proj_finish(c):
                csl = slice(c * 128, (c + 1) * 128)
                ps_f = proj_state.pop(c)
                nc.tensor.matmul(
                    ps_f, lhsT=ot_sb[3][:, csl], rhs=wt_sb[:, 3, :],
                    start=False, stop=True,
                )
                f_sb = ptp.tile([128, D], bf, tag="fin", name="f_sb")
                if c % 2 == 0:
                    nc.vector.tensor_copy(f_sb, ps_f)
                else:
                    nc.scalar.copy(f_sb, ps_f)
                # alternate output-DMA queues: the SP SEQ costs ~0.65us per
                # issue and head-of-line blocks on the copy, so the idle
                # Pool (SWDGE) queue takes every other chunk
                if c % 2 == 0:
                    nc.sync.dma_start(out_d[csl, :], f_sb)
                else:
                    nc.gpsimd.dma_start(out_d[csl, :], f_sb)

            def emit_proj(c, tag):
                emit_proj_partial(c, tag)
                emit_proj_finish(c)

            def emit_tail_half(ih):
                # transposes of the half's 4 slices, copy to SBUF, then the
                # matching projection chunks
                st = pair_state[3]
                on = st["on"]
                if ih == 0:
                    tpl = ps_s.tile([128, 512], f32, tag="sb0", name="tp_last")
                    st["tpl"] = tpl
                    tgt = tpl[:].bitcast(bf)
                else:
                    tgt = st["sum"][:].bitcast(bf)
                for sl in range(ih * 4, ih * 4 + 4):
                    nc.tensor.matmul(
                        tgt[:, (sl % 4) * 128 : (sl % 4) * 128 + 128]
                        if ih == 0
                        else tgt[:, sl * 128 : sl * 128 + 128],
                        lhsT=on[:, sl, :],
                        rhs=idn_sb,
                        is_transpose=True,
                        start=(sl % 4 == 0),
                        stop=(sl % 4 == 3),
                    )
                src_cols = slice(0, 512) if ih == 0 else slice(512, 1024)
                if ih == 0:
                    nc.vector.tensor_copy(
                        ot_sb[3][:, 0:512], tgt[:, src_cols]
                    )
                    for c, tag in ((0, "sa0"), (1, "sa1"), (2, "sb2"), (3, "sa0")):
                        emit_proj(c, tag)
                    for c, tag in ((4, "sb1"), (5, "sa1"), (6, "sb2"), (7, "sa0")):
                        emit_proj_partial(c, tag)
                else:
                    nc.scalar.copy(ot_sb[3][:, 512:1024], tgt[:, src_cols])
                    for c in range(4, 8):
                        emit_proj_finish(c)

            def emit_transpose(hp):
                # The sum bank is dead after the normalize reads it, so the
                # pair's 8 transposes reuse it (one accumulation group, each
                # byte written exactly once), then a DMA moves O^T to SBUF —
                # no engine cycles spent on the copy.
                st = pair_state[hp]
                on = st["on"]
                tp = st["sum"][:].bitcast(bf)
                for sl in range(8):
                    nc.tensor.matmul(
                        tp[:, sl * 128 : (sl + 1) * 128],
                        lhsT=on[:, sl, :],
                        rhs=idn_sb,
                        is_transpose=True,
                        start=(sl == 0),
                        stop=(sl == 7),
                    )
                if hp == 3:
                    nc.scalar.copy(ot_sb[hp], tp)
                else:
                    nc.vector.tensor_copy(ot_sb[hp], tp)

            def alloc_pair(hp):
                pair_state[hp] = {
                    "o": [
                        ps_o.tile([128, 8, HD], f32, tag=f"o{h01}", name=f"o{h01}")
                        for h01 in range(2)
                    ],
                    "sum": ps_o.tile([128, 512], f32, tag="sum", name="sum"),
                    "rc": work.tile([128, 16], f32, tag="rc", name="rc"),
                    "on": work.tile([128, 8, 128], bf, tag="on", name="on"),
                }
                if hp - 2 in pair_state:
                    del pair_state[hp - 2]

            # Software-pipelined emission, PV lagging QK/exp by LAG units so
            # the PE never waits on the exp engines in steady state. Pair
            # boundary work is interleaved: norm right after the pair's last
            # PV, transposes one iteration later (so the PE meets them after
            # the DVE normalize has finished), all before the next pair
            # reuses the same psum tags.
            LAG = 5
            units = [(hp, jt, ih) for hp in range(4) for jt in range(JT) for ih in range(NU)]
            transposes = []  # (due_iter, hp)
            tails = []  # (due_iter, ih) for the last pair

            def emit_iter(u):
                # PV of u-LAG first: its inputs are ready, so the in-order
                # PE SEQ does useful work while QK(u) waits out the s-buffer
                # WAR on the exp engines of u-2.
                pu = u - LAG
                if 0 <= pu < len(units):
                    php, pjt, pih = units[pu]
                    if pjt == 0 and pih == 0:
                        alloc_pair(php)
                    emit_pv(php, pjt, pih, pending_p[pu])
                    pending_p[pu] = None
                # norm before this unit's exp: it then sits ahead of the
                # DVE's next bit-exp in the queue, starting the pair-close
                # chain one exp earlier (sb's 3-deep buffering absorbs the
                # delayed exp)
                if 0 <= pu < len(units):
                    php, pjt, pih = units[pu]
                    if pjt == JT - 1:
                        if php == 3:
                            emit_norm_half(pih)
                            tails.append((u + 1, pih))
                        elif pih == 1:
                            emit_norm(php)
                            transposes.append((u, php))
                if u < len(units):
                    hp, jt, ih = units[u]
                    emit_qk_exp(hp, jt, ih, u)
                while transposes and transposes[0][0] <= u:
                    _, thp = transposes.pop(0)
                    emit_transpose(thp)
                while tails and tails[0][0] <= u:
                    _, tih = tails.pop(0)
                    emit_tail_half(tih)

            pending_p = {}
            orig_qk = emit_qk_exp

            def emit_qk_exp_wrap(hp, jt, ih, u):
                pending_p[u] = orig_qk(hp, jt, ih, u)

            emit_qk_exp = emit_qk_exp_wrap
            for u in range(len(units) + LAG):
                emit_iter(u)
            while transposes:
                _, thp = transposes.pop(0)
                emit_transpose(thp)
            while tails:
                _, tih = tails.pop(0)
                emit_tail_half(tih)


    nc.finalize()
    return nc


def _host_prep(q, k, v, W_out, b_out):
    """Shard + lay out inputs per core (pure layout: transpose/pack/bf16)."""
    import ml_dtypes

    bf16 = ml_dtypes.bfloat16
    q = np.asarray(q, dtype=np.float32)
    k = np.asarray(k, dtype=np.float32)
    v = np.asarray(v, dtype=np.float32)
    W_out = np.asarray(W_out, dtype=np.float32)
    b_out = np.asarray(b_out, dtype=np.float32)

    qT = np.ascontiguousarray(q.transpose(0, 2, 1)).astype(bf16)  # [B, D, N]
    kT = np.ascontiguousarray(k.transpose(0, 2, 1)).astype(bf16)  # [B, D, M]
    # va[p, jt, h, hd] = v[b, jt*128 + p, h*64 + hd]
    va = np.ascontiguousarray(
        v.reshape(B, JT, 128, H, HD).transpose(0, 2, 1, 3, 4)
    ).astype(bf16)
    wt = np.ascontiguousarray(W_out.T.reshape(4, 128, D)).astype(bf16)
    idn = np.eye(128, dtype=np.float32).astype(bf16)

    in_maps = []
    for c in range(NCORES):
        b_, ihalf = divmod(c, 2)
        in_maps.append(
            {
                "qt": np.ascontiguousarray(
                    qT[b_, :, ihalf * NLOC : (ihalf + 1) * NLOC].reshape(4, 128, NLOC)
                ),
                "kt": np.ascontiguousarray(kT[b_].reshape(4, 128, M)),
                "va": va[b_],
                "wt": wt,
                "idn": idn,
            }
        )
    return in_maps


def kernel(q, k, v, W_out, b_out):
    from concourse.bass_utils import run_bass_kernel_spmd

    nc = _build_bass()
    in_maps = _host_prep(q, k, v, W_out, b_out)
    res = run_bass_kernel_spmd(nc, in_maps, core_ids=list(range(NCORES)))
    out = np.empty((B, N, D), dtype=np.float32)
    for c, r_ in enumerate(res.results):
        b_, ihalf = divmod(c, 2)
        out[b_, ihalf * NLOC : (ihalf + 1) * NLOC, :] = r_["out"].astype(
            np.float32
        )
    b_vec = np.asarray(b_out, dtype=np.float32)
    if b_vec.any():
        out += b_vec[None, None, :]
    return out


# revision 68
# speedup vs baseline: 1.0015x; 1.0007x over previous
"""Multi-head attention + output projection on 8 Trainium2 NeuronCores.

Problem (hardcoded): B=4, N=M=2048, D_IN=D_OUT=512, H=8, HD=VD=64.
  out = softmax(q @ k^T / sqrt(64)) @ v, heads concat, @ W_out.T + b_out

Sharding: each core owns (batch b = core//2, query-row half = core%2):
  q-chunk [1024, 512], full k/v for that batch, full W_out. All 8 heads are
  computed locally, so no collectives are needed; the host concatenates the
  8 disjoint [1024, 512] output chunks.

Device algorithm per core — engine-balanced around four ideas:
  1. exp split across ACT and DVE with per-head S tiles: each unit
     (pair, jt, i-half) produces two [128, 512] S psum tiles, one per
     head. ACT exps head0 natively (bf16 out); DVE exps head1 with a
     Schraudolph bit-trick — one tensor_scalar i16(S*c1+c2) whose int16
     result IS the bf16 bit pattern of 2^(0.125*S*log2e) (~1.8% RMS on
     those columns; the softmax ratio cancels the mean part, ~1.3%
     end-to-end vs the 2e-2 gate). Every 17th unit ACT takes head1 too,
     keeping both engines at the PE's pace. Each psum tile has exactly
     ONE reader engine: the tile framework serializes cross-engine PSUM
     readers, and GPSIMD cannot touch PSUM at all (so Pool sits out).
  2. PV uses P^T as the *stationary* matmul operand (lhsT) and the 64-col
     v tile as the moving one, so each accumulation step bills only 64
     columns instead of 1024: O[i,d] psum accumulates per 128-i-slice
     over the 16 j-tiles; sumexp rides as 1-col ones matmuls. PSUM
     accumulation groups are per 2KB bank: one start/stop per o-bank and
     sum-bank per pair (start lazily zero-marks the bank; each byte is
     claimed once). Normalization = DVE recip + broadcast multiply; a PE
     transpose chain (8x [128,128] bf16, one group) writes O^T into the
     dead sum bank, freeing a PSUM bank that gives head1's S tiles
     3-deep buffering.
  3. bf16 everywhere off the f32 PSUM accumulators (q/k/v/P/W_out),
     halving DMA and enabling the small-free-dim matmuls at 1 cyc/col.
     b_out is added on the host (it is zeros here anyway).
  4. Software-pipelined emission: PV lags QK/exp by LAG=4 units so the
     in-order PE SEQ never parks on an exp-produced weight; pair-boundary
     transposes are deferred one unit. The last pair closes its slice 0-3
     accumulation groups one unit early (ih1 writes run with the group
     check skipped, exact via the pending-zero marks), so normalize /
     transpose / projection of the first i-half and the pair-0..2 partial
     projections of the second overlap the final unit's compute.
  Output is bf16 (host converts to f32 and adds b_out); the eight output
  DMAs alternate between the SP and Pool (SWDGE) queues to halve the
  serial issue cost on the tail.
  Cost model: ~105.2us/core (PE 91us busy: QK 131072 cyc + PV 66560 +
  transposes + 16384 proj; ACT ~88us; DVE ~90us), rel err ~1.3e-2.
  Remaining idle is latency-bound: ~3us first-DMA fixed costs (SEQ/DGE/
  sem-prop), ~3.5us s-buffer WAR chain jitter at ~0.08us/2-unit margin,
  ~5us closing chain (last PV -> norm -> transpose -> proj -> copy ->
  DMA -> drain).
"""

import numpy as np

B, N, M, D, H, HD = 4, 2048, 2048, 512, 8, 64
NLOC = N // 2  # query rows per core
NCORES = 8
JT = M // 128  # 16 j-tiles
NU = 2  # i-halves per jt (512 cols each)
LOG2E = 1.4426950408889634
C_SCH = 0.0573  # Schraudolph bias minimizing RMS rel err of the bf16 bit-exp
C1 = float(np.float32(0.125 * LOG2E * 128.0))
C2 = float(np.float32((127.0 - C_SCH) * 128.0 + 0.5))  # +0.5: i16 cast truncates


def _build_bass(debug=False):
    import concourse.mybir as mybir
    import concourse.tile as tile
    from concourse import bacc

    f32 = mybir.dt.float32
    bf = mybir.dt.bfloat16
    i16 = mybir.dt.int16
    Exp = mybir.ActivationFunctionType.Exp
    mult = mybir.AluOpType.mult
    add = mybir.AluOpType.add

    nc = bacc.Bacc()
    qt_d = nc.dram_tensor("qt", [4, 128, NLOC], bf, kind="ExternalInput")
    kt_d = nc.dram_tensor("kt", [4, 128, M], bf, kind="ExternalInput")
    va_d = nc.dram_tensor("va", [128, JT, H, HD], bf, kind="ExternalInput")
    wt_d = nc.dram_tensor("wt", [4, 128, D], bf, kind="ExternalInput")
    idn_d = nc.dram_tensor("idn", [128, 128], bf, kind="ExternalInput")
    out_d = nc.dram_tensor("out", [NLOC, D], bf, kind="ExternalOutput")
    if debug:
        dbg = {
            "dbg_p": nc.dram_tensor("dbg_p", [128, 2, 512], f32, kind="ExternalOutput"),
            "dbg_o": nc.dram_tensor("dbg_o", [128, 8, HD], f32, kind="ExternalOutput"),
            "dbg_sm": nc.dram_tensor("dbg_sm", [128, 16], f32, kind="ExternalOutput"),
            "dbg_on": nc.dram_tensor("dbg_on", [128, 8, 128], f32, kind="ExternalOutput"),
            "dbg_ot": nc.dram_tensor("dbg_ot", [128, NLOC], f32, kind="ExternalOutput"),
        }

    with tile.TileContext(nc) as tc:
        with (
            tc.tile_pool(name="persist", bufs=1) as persist,
            tc.tile_pool(name="pt", bufs=30) as ptp,
            tc.tile_pool(name="work", bufs=2) as work,
            tc.tile_pool(name="ps_s", bufs=1, space="PSUM") as ps_s,
            tc.tile_pool(name="ps_o", bufs=1, space="PSUM") as ps_o,
        ):
            kt_sb = [persist.tile([128, M], bf, tag=f"kt{o}", name=f"kt{o}") for o in range(4)]
            qt_sb = [persist.tile([128, NLOC], bf, tag=f"qt{o}", name=f"qt{o}") for o in range(4)]
            va_sb = persist.tile([128, JT, H, HD], bf)
            wt_sb = persist.tile([128, 4, D], bf)
            idn_sb = persist.tile([128, 128], bf)
            ones_c = persist.tile([128, 1], bf)
            ot_sb = [persist.tile([128, NLOC], bf, tag=f"ot{o}", name=f"ot{o}") for o in range(4)]

            nc.vector.memset(ones_c, 1.0)
            # PE pstate warmup: ~3us of dummy matmuls into the (not yet
            # used) sum bank so the clock ramp finishes before the first
            # real QK; the first pair's sum generation waits on the WAW and
            # starts well after these complete.
            dumw = persist.tile([128, 512], bf)
            nc.vector.memset(dumw, 1.0)
            dum_ps = ps_o.tile([128, 512], f32, tag="sum", name="dum_ps")
            for _ in range(0):
                nc.tensor.matmul(
                    dum_ps, lhsT=dumw[:, 0:128], rhs=dumw, start=True, stop=True
                )

            # DMA schedule ordered by first use: first QK needs kt0 jt0 +
            # qt0 half0; first PV needs va jt0; then stream the rest.
            nc.sync.dma_start(kt_sb[0][:, 0:128], kt_d[0, :, 0:128])
            nc.scalar.dma_start(qt_sb[0][:, 0:512], qt_d[0, :, 0:512])
            nc.sync.dma_start(qt_sb[0][:, 512:1024], qt_d[0, :, 512:1024])
            nc.gpsimd.dma_start(kt_sb[0][:, 128:512], kt_d[0, :, 128:512])
            nc.scalar.dma_start(va_sb[:, 0:1], va_d[:, 0:1])
            nc.sync.dma_start(va_sb[:, 1:4], va_d[:, 1:4])
            nc.sync.dma_start(kt_sb[0][:, 512:2048], kt_d[0, :, 512:2048])
            nc.sync.dma_start(va_sb[:, 4:10], va_d[:, 4:10])
            nc.sync.dma_start(va_sb[:, 10:16], va_d[:, 10:16])
            for o in range(1, 4):
                nc.sync.dma_start(kt_sb[o], kt_d[o])
                nc.sync.dma_start(qt_sb[o], qt_d[o])
            nc.sync.dma_start(idn_sb, idn_d[:])
            for o in range(4):
                nc.sync.dma_start(wt_sb[:, o : o + 1], wt_d[o : o + 1])

            # per-pair psum/staging tiles, captured per pair generation so
            # deferred work (normalize/transpose) reads the right tiles
            pair_state = {}

            def emit_qk_exp(hp, jt, ih, u):
                # One PSUM tile per head per unit, each with exactly ONE
                # reader engine: the tile framework serializes cross-engine
                # READERS of a PSUM tile (reader chaining), so sharing one S
                # tile between ACT and DVE would run them back-to-back.
                buf = u % 2
                s0 = ps_s.tile([128, 512], f32, tag=f"sa{buf}", name="s0")
                s1 = ps_s.tile([128, 512], f32, tag=f"sb{u % 3}", name="s1")
                for h01, st in ((0, s0), (1, s1)):
                    nc.tensor.matmul(
                        st,
                        lhsT=kt_sb[hp][64 * h01 : 64 * h01 + 64, jt * 128 : (jt + 1) * 128],
                        rhs=qt_sb[hp][64 * h01 : 64 * h01 + 64, ih * 512 : (ih + 1) * 512],
                        start=True,
                        stop=True,
                    )
                # GPSIMD cannot read PSUM on real HW, so only ACT and DVE
                # can consume S. DVE does head1's bit-trick exp on most
                # units; every 9th unit ACT takes head1 too (native exp) to
                # keep the DVE total under the PE-work bound.
                pa = ptp.tile([128, 512], bf, tag=f"pa{buf}", name="pa")
                nc.scalar.activation(pa, s0, Exp, scale=0.125)
                px = ptp.tile([128, 512], bf, tag=f"px{buf}", name="px")
                if u % 17 == 5:
                    # ACT absorbs head1 too on this cadence to balance DVE
                    nc.scalar.activation(px, s1, Exp, scale=0.125)
                else:
                    nc.vector.tensor_scalar(
                        px[:].bitcast(i16), s1, C1, C2, mult, add
                    )
                return (pa, px)

            def emit_pv(hp, jt, ih, p):
                o_tiles, sum_ps = pair_state[hp]["o"], pair_state[hp]["sum"]
                pa, px = p
                slices = [
                    [pa[:, 0:128], pa[:, 128:256], pa[:, 256:384], pa[:, 384:512]],
                    [px[:, 0:128], px[:, 128:256], px[:, 256:384], px[:, 384:512]],
                ]
                # PSUM accumulation groups are per 2KB zero-region (bank):
                # exactly one start (first touch zero-marks the whole bank)
                # and one stop (last touch) per o-bank / sum-bank per pair.
                # Last pair: slices 0-3 form their own accumulation group
                # that closes at (jt15, ih0), so normalize/transpose/proj of
                # the first i-half overlaps the final i-half's compute. The
                # ih1 writes keep accumulating with the group check skipped
                # (their bytes still carry the pending-zero marks from the
                # group-A start, so values stay exact).
                last = hp == 3
                for h01 in range(2):
                    for sl in range(4):
                        gsl = ih * 4 + sl
                        lhsT = slices[h01][sl]
                        o_stop = (
                            (jt == JT - 1 and gsl in (3, 7))
                            if last
                            else (jt == JT - 1 and gsl == 7)
                        )
                        nc.tensor.matmul(
                            o_tiles[h01][:, gsl, :],
                            lhsT=lhsT,
                            rhs=va_sb[:, jt, 2 * hp + h01, :],
                            start=(jt == 0 and gsl == 0),
                            stop=o_stop,
                            skip_group_check=(last and ih == 1),
                        )
                        s_stop = (
                            (jt == JT - 1 and gsl in (3, 7) and h01 == 1)
                            if last
                            else (jt == JT - 1 and gsl == 7 and h01 == 1)
                        )
                        nc.tensor.matmul(
                            sum_ps[:, h01 * 8 + gsl : h01 * 8 + gsl + 1],
                            lhsT=lhsT,
                            rhs=ones_c,
                            start=(jt == 0 and gsl == 0 and h01 == 0),
                            stop=s_stop,
                            skip_group_check=(last and ih == 1),
                        )

            def emit_norm(hp):
                # softmax normalization for the whole pair (PSUM accumulation
                # groups close at the pair's last PV, and mid-group reads are
                # not allowed): recip of sumexp, broadcast-multiply, bf16 out
                # staged for transpose
                st = pair_state[hp]
                o_tiles, sum_ps = st["o"], st["sum"]
                rc, on = st["rc"], st["on"]
                nc.vector.reciprocal(rc, sum_ps[:, 0:16])
                for h01 in range(2):
                    nc.vector.tensor_tensor(
                        on[:, :, 64 * h01 : 64 * h01 + 64],
                        o_tiles[h01],
                        rc[:, h01 * 8 : h01 * 8 + 8].unsqueeze(2).broadcast_to(
                            [128, 8, HD]
                        ),
                        mult,
                    )
                if debug and hp == 0:
                    dbg_o = work.tile([128, 8, HD], f32, tag="dbg_o", name="dbg_o")
                    nc.vector.tensor_copy(dbg_o, o_tiles[0])
                    nc.sync.dma_start(dbg["dbg_o"][:], dbg_o)

            def emit_norm_half(ih):
                st = pair_state[3]
                o_tiles, sum_ps = st["o"], st["sum"]
                rc, on = st["rc"], st["on"]
                a = ih * 4
                for h01 in range(2):
                    nc.vector.reciprocal(
                        rc[:, h01 * 8 + a : h01 * 8 + a + 4],
                        sum_ps[:, h01 * 8 + a : h01 * 8 + a + 4],
                    )
                    nc.vector.tensor_tensor(
                        on[:, a : a + 4, 64 * h01 : 64 * h01 + 64],
                        o_tiles[h01][:, a : a + 4, :],
                        rc[:, h01 * 8 + a : h01 * 8 + a + 4]
                        .unsqueeze(2)
                        .broadcast_to([128, 4, HD]),
                        mult,
                    )

            proj_state = {}

            def emit_proj_partial(c, tag):
                # pairs 0-2 of chunk c (group left open; finished later once
                # ot3 lands). Banks: the sa/sb psum tags free up as the last
                # units retire, giving four chunks in flight.
                csl = slice(c * 128, (c + 1) * 128)
                ps_f = ps_s.tile([128, D], f32, tag=tag, name="ps_f")
                proj_state[c] = ps_f
                for o in range(3):
                    nc.tensor.matmul(
                        ps_f,
                        lhsT=ot_sb[o][:, csl],
                        rhs=wt_sb[:, o, :],
                        start=(o == 0),
                        stop=False,
                    )

            def emit_proj_finish(c):
                csl = slice(c * 128, (c + 1) * 128)
                ps_f = proj_state.pop(c)
                nc.tensor.matmul(
                    ps_f, lhsT=ot_sb[3][:, csl], rhs=wt_sb[:, 3, :],
                    start=False, stop=True,
                )
                f_sb = ptp.tile([128, D], bf, tag="fin", name="f_sb")
                if c % 2 == 0:
                    nc.vector.tensor_copy(f_sb, ps_f)
                else:
                    nc.scalar.copy(f_sb, ps_f)
                # alternate output-DMA queues: the SP SEQ costs ~0.65us per
                # issue and head-of-line blocks on the copy, so the idle
                # Pool (SWDGE) queue takes every other chunk
                if c % 2 == 0:
                    nc.sync.dma_start(out_d[csl, :], f_sb)
                else:
                    nc.gpsimd.dma_start(out_d[csl, :], f_sb)

            def emit_proj(c, tag):
                emit_proj_partial(c, tag)
                emit_proj_finish(c)

            def emit_tail_half(ih):
                # transposes of the half's 4 slices, copy to SBUF, then the
                # matching projection chunks
                st = pair_state[3]
                on = st["on"]
                if ih == 0:
                    tpl = ps_s.tile([128, 512], f32, tag="sb0", name="tp_last")
                    st["tpl"] = tpl
                    tgt = tpl[:].bitcast(bf)
                else:
                    tgt = st["sum"][:].bitcast(bf)
                for sl in range(ih * 4, ih * 4 + 4):
                    nc.tensor.matmul(
                        tgt[:, (sl % 4) * 128 : (sl % 4) * 128 + 128]
                        if ih == 0
                        else tgt[:, sl * 128 : sl * 128 + 128],
                        lhsT=on[:, sl, :],
                        rhs=idn_sb,
                        is_transpose=True,
                        start=(sl % 4 == 0),
                        stop=(sl % 4 == 3),
                    )
                src_cols = slice(0, 512) if ih == 0 else slice(512, 1024)
                if ih == 0:
                    nc.vector.tensor_copy(
                        ot_sb[3][:, 0:512], tgt[:, src_cols]
                    )
                    for c, tag in ((0, "sa0"), (1, "sa1"), (2, "sb2"), (3, "sa0")):
                        emit_proj(c, tag)
                    for c, tag in ((4, "sb1"), (5, "sa1"), (6, "sb2"), (7, "sa0")):
                        emit_proj_partial(c, tag)
                else:
                    nc.scalar.copy(ot_sb[3][:, 512:1024], tgt[:, src_cols])
                    for c in range(4, 8):
                        emit_proj_finish(c)

            def emit_transpose(hp):
                # The sum bank is dead after the normalize reads it, so the
                # pair's 8 transposes reuse it (one accumulation group, each
                # byte written exactly once), then a DMA moves O^T to SBUF —
                # no engine cycles spent on the copy.
                st = pair_state[hp]
                on = st["on"]
                tp = st["sum"][:].bitcast(bf)
                for sl in range(8):
                    nc.tensor.matmul(
                        tp[:, sl * 128 : (sl + 1) * 128],
                        lhsT=on[:, sl, :],
                        rhs=idn_sb,
                        is_transpose=True,
                        start=(sl == 0),
                        stop=(sl == 7),
                    )
                if hp == 3:
                    nc.scalar.copy(ot_sb[hp], tp)
                else:
                    nc.vector.tensor_copy(ot_sb[hp], tp)

            def alloc_pair(hp):
                pair_state[hp] = {
                    "o": [
                        ps_o.tile([128, 8, HD], f32, tag=f"o{h01}", name=f"o{h01}")
                        for h01 in range(2)
                    ],
                    "sum": ps_o.tile([128, 512], f32, tag="sum", name="sum"),
                    "rc": work.tile([128, 16], f32, tag="rc", name="rc"),
                    "on": work.tile([128, 8, 128], bf, tag="on", name="on"),
                }
                if hp - 2 in pair_state:
                    del pair_state[hp - 2]

            # Software-pipelined emission, PV lagging QK/exp by LAG units so
            # the PE never waits on the exp engines in steady state. Pair
            # boundary work is interleaved: norm right after the pair's last
            # PV, transposes one iteration later (so the PE meets them after
            # the DVE normalize has finished), all before the next pair
            # reuses the same psum tags.
            LAG = 5
            units = [(hp, jt, ih) for hp in range(4) for jt in range(JT) for ih in range(NU)]
            transposes = []  # (due_iter, hp)
            tails = []  # (due_iter, ih) for the last pair

            def emit_iter(u):
                # PV of u-LAG first: its inputs are ready, so the in-order
                # PE SEQ does useful work while QK(u) waits out the s-buffer
                # WAR on the exp engines of u-2.
                pu = u - LAG
                if 0 <= pu < len(units):
                    php, pjt, pih = units[pu]
                    if pjt == 0 and pih == 0:
                        alloc_pair(php)
                    emit_pv(php, pjt, pih, pending_p[pu])
                    pending_p[pu] = None
                # norm before this unit's exp: it then sits ahead of the
                # DVE's next bit-exp in the queue, starting the pair-close
                # chain one exp earlier (sb's 3-deep buffering absorbs the
                # delayed exp)
                if 0 <= pu < len(units):
                    php, pjt, pih = units[pu]
                    if pjt == JT - 1:
                        if php == 3:
                            emit_norm_half(pih)
                            tails.append((u + 1, pih))
                        elif pih == 1:
                            emit_norm(php)
                            transposes.append((u, php))
                if u < len(units):
                    hp, jt, ih = units[u]
                    emit_qk_exp(hp, jt, ih, u)
                while transposes and transposes[0][0] <= u:
                    _, thp = transposes.pop(0)
                    emit_transpose(thp)
                while tails and tails[0][0] <= u:
                    _, tih = tails.pop(0)
                    emit_tail_half(tih)

            pending_p = {}
            orig_qk = emit_qk_exp

            def emit_qk_exp_wrap(hp, jt, ih, u):
                pending_p[u] = orig_qk(hp, jt, ih, u)

            emit_qk_exp = emit_qk_exp_wrap
            for u in range(len(units) + LAG):
                emit_iter(u)
            while transposes:
                _, thp = transposes.pop(0)
                emit_transpose(thp)
            while tails:
                _, tih = tails.pop(0)
                emit_tail_half(tih)


    nc.finalize()
    return nc


def _host_prep(q, k, v, W_out, b_out):
    """Shard + lay out inputs per core (pure layout: transpose/pack/bf16)."""
    import ml_dtypes

    bf16 = ml_dtypes.bfloat16
    q = np.asarray(q, dtype=np.float32)
    k = np.asarray(k, dtype=np.float32)
    v = np.asarray(v, dtype=np.float32)
    W_out = np.asarray(W_out, dtype=np.float32)
    b_out = np.asarray(b_out, dtype=np.float32)

    qT = np.ascontiguousarray(q.transpose(0, 2, 1)).astype(bf16)  # [B, D, N]
    kT = np.ascontiguousarray(k.transpose(0, 2, 1)).astype(bf16)  # [B, D, M]
    # va[p, jt, h, hd] = v[b, jt*128 + p, h*64 + hd]
    va = np.ascontiguousarray(
        v.reshape(B, JT, 128, H, HD).transpose(0, 2, 1, 3, 4)
    ).astype(bf16)
    wt = np.ascontiguousarray(W_out.T.reshape(4, 128, D)).astype(bf16)
    idn = np.eye(128, dtype=np.float32).astype(bf16)

    in_maps = []
    for c in range(NCORES):
        b_, ihalf = divmod(c, 2)
        in_maps.append(
            {
                "qt": np.ascontiguousarray(
                    qT[b_, :, ihalf * NLOC : (ihalf + 1) * NLOC].reshape(4, 128, NLOC)
                ),
                "kt": np.ascontiguousarray(kT[b_].reshape(4, 128, M)),
                "va": va[b_],
                "wt": wt,
                "idn": idn,
            }
        )
    return in_maps


def kernel(q, k, v, W_out, b_out):
    from concourse.bass_utils import run_bass_kernel_spmd

    nc = _build_bass()
    in_maps = _host_prep(q, k, v, W_out, b_out)
    res = run_bass_kernel_spmd(nc, in_maps, core_ids=list(range(NCORES)))
    out = np.empty((B, N, D), dtype=np.float32)
    for c, r_ in enumerate(res.results):
        b_, ihalf = divmod(c, 2)
        out[b_, ihalf * NLOC : (ihalf + 1) * NLOC, :] = r_["out"].astype(
            np.float32
        )
    b_vec = np.asarray(b_out, dtype=np.float32)
    if b_vec.any():
        out += b_vec[None, None, :]
    return out


# revision 69
# speedup vs baseline: 1.0017x; 1.0002x over previous
"""Multi-head attention + output projection on 8 Trainium2 NeuronCores.

Problem (hardcoded): B=4, N=M=2048, D_IN=D_OUT=512, H=8, HD=VD=64.
  out = softmax(q @ k^T / sqrt(64)) @ v, heads concat, @ W_out.T + b_out

Sharding: each core owns (batch b = core//2, query-row half = core%2):
  q-chunk [1024, 512], full k/v for that batch, full W_out. All 8 heads are
  computed locally, so no collectives are needed; the host concatenates the
  8 disjoint [1024, 512] output chunks.

Device algorithm per core — engine-balanced around four ideas:
  1. exp split across ACT and DVE with per-head S tiles: each unit
     (pair, jt, i-half) produces two [128, 512] S psum tiles, one per
     head. ACT exps head0 natively (bf16 out); DVE exps head1 with a
     Schraudolph bit-trick — one tensor_scalar i16(S*c1+c2) whose int16
     result IS the bf16 bit pattern of 2^(0.125*S*log2e) (~1.8% RMS on
     those columns; the softmax ratio cancels the mean part, ~1.3%
     end-to-end vs the 2e-2 gate). Every 17th unit ACT takes head1 too,
     keeping both engines at the PE's pace. Each psum tile has exactly
     ONE reader engine: the tile framework serializes cross-engine PSUM
     readers, and GPSIMD cannot touch PSUM at all (so Pool sits out).
  2. PV uses P^T as the *stationary* matmul operand (lhsT) and the 64-col
     v tile as the moving one, so each accumulation step bills only 64
     columns instead of 1024: O[i,d] psum accumulates per 128-i-slice
     over the 16 j-tiles; sumexp rides as 1-col ones matmuls. PSUM
     accumulation groups are per 2KB bank: one start/stop per o-bank and
     sum-bank per pair (start lazily zero-marks the bank; each byte is
     claimed once). Normalization = DVE recip + broadcast multiply; a PE
     transpose chain (8x [128,128] bf16, one group) writes O^T into the
     dead sum bank, freeing a PSUM bank that gives head1's S tiles
     3-deep buffering.
  3. bf16 everywhere off the f32 PSUM accumulators (q/k/v/P/W_out),
     halving DMA and enabling the small-free-dim matmuls at 1 cyc/col.
     b_out is added on the host (it is zeros here anyway).
  4. Software-pipelined emission: PV lags QK/exp by LAG=4 units so the
     in-order PE SEQ never parks on an exp-produced weight; pair-boundary
     transposes are deferred one unit. The last pair closes its slice 0-3
     accumulation groups one unit early (ih1 writes run with the group
     check skipped, exact via the pending-zero marks), so normalize /
     transpose / projection of the first i-half and the pair-0..2 partial
     projections of the second overlap the final unit's compute.
  Output is bf16 (host converts to f32 and adds b_out); the eight output
  DMAs alternate between the SP and Pool (SWDGE) queues to halve the
  serial issue cost on the tail.
  Cost model: ~105.2us/core (PE 91us busy: QK 131072 cyc + PV 66560 +
  transposes + 16384 proj; ACT ~88us; DVE ~90us), rel err ~1.3e-2.
  Remaining idle is latency-bound: ~3us first-DMA fixed costs (SEQ/DGE/
  sem-prop), ~3.5us s-buffer WAR chain jitter at ~0.08us/2-unit margin,
  ~5us closing chain (last PV -> norm -> transpose -> proj -> copy ->
  DMA -> drain).
"""

import numpy as np

B, N, M, D, H, HD = 4, 2048, 2048, 512, 8, 64
NLOC = N // 2  # query rows per core
NCORES = 8
JT = M // 128  # 16 j-tiles
NU = 2  # i-halves per jt (512 cols each)
LOG2E = 1.4426950408889634
C_SCH = 0.0573  # Schraudolph bias minimizing RMS rel err of the bf16 bit-exp
C1 = float(np.float32(0.125 * LOG2E * 128.0))
C2 = float(np.float32((127.0 - C_SCH) * 128.0 + 0.5))  # +0.5: i16 cast truncates


def _build_bass(debug=False):
    import concourse.mybir as mybir
    import concourse.tile as tile
    from concourse import bacc

    f32 = mybir.dt.float32
    bf = mybir.dt.bfloat16
    i16 = mybir.dt.int16
    Exp = mybir.ActivationFunctionType.Exp
    mult = mybir.AluOpType.mult
    add = mybir.AluOpType.add

    nc = bacc.Bacc()
    qt_d = nc.dram_tensor("qt", [4, 128, NLOC], bf, kind="ExternalInput")
    kt_d = nc.dram_tensor("kt", [4, 128, M], bf, kind="ExternalInput")
    va_d = nc.dram_tensor("va", [128, JT, H, HD], bf, kind="ExternalInput")
    wt_d = nc.dram_tensor("wt", [4, 128, D], bf, kind="ExternalInput")
    idn_d = nc.dram_tensor("idn", [128, 128], bf, kind="ExternalInput")
    out_d = nc.dram_tensor("out", [NLOC, D], bf, kind="ExternalOutput")
    if debug:
        dbg = {
            "dbg_p": nc.dram_tensor("dbg_p", [128, 2, 512], f32, kind="ExternalOutput"),
            "dbg_o": nc.dram_tensor("dbg_o", [128, 8, HD], f32, kind="ExternalOutput"),
            "dbg_sm": nc.dram_tensor("dbg_sm", [128, 16], f32, kind="ExternalOutput"),
            "dbg_on": nc.dram_tensor("dbg_on", [128, 8, 128], f32, kind="ExternalOutput"),
            "dbg_ot": nc.dram_tensor("dbg_ot", [128, NLOC], f32, kind="ExternalOutput"),
        }

    with tile.TileContext(nc) as tc:
        with (
            tc.tile_pool(name="persist", bufs=1) as persist,
            tc.tile_pool(name="pt", bufs=36) as ptp,
            tc.tile_pool(name="work", bufs=2) as work,
            tc.tile_pool(name="ps_s", bufs=1, space="PSUM") as ps_s,
            tc.tile_pool(name="ps_o", bufs=1, space="PSUM") as ps_o,
        ):
            kt_sb = [persist.tile([128, M], bf, tag=f"kt{o}", name=f"kt{o}") for o in range(4)]
            qt_sb = [persist.tile([128, NLOC], bf, tag=f"qt{o}", name=f"qt{o}") for o in range(4)]
            va_sb = persist.tile([128, JT, H, HD], bf)
            wt_sb = persist.tile([128, 4, D], bf)
            idn_sb = persist.tile([128, 128], bf)
            ones_c = persist.tile([128, 1], bf)
            ot_sb = [persist.tile([128, NLOC], bf, tag=f"ot{o}", name=f"ot{o}") for o in range(4)]

            nc.vector.memset(ones_c, 1.0)
            # PE pstate warmup: ~3us of dummy matmuls into the (not yet
            # used) sum bank so the clock ramp finishes before the first
            # real QK; the first pair's sum generation waits on the WAW and
            # starts well after these complete.
            dumw = persist.tile([128, 512], bf)
            nc.vector.memset(dumw, 1.0)
            dum_ps = ps_o.tile([128, 512], f32, tag="sum", name="dum_ps")
            for _ in range(0):
                nc.tensor.matmul(
                    dum_ps, lhsT=dumw[:, 0:128], rhs=dumw, start=True, stop=True
                )

            # DMA schedule ordered by first use: first QK needs kt0 jt0 +
            # qt0 half0; first PV needs va jt0; then stream the rest.
            nc.sync.dma_start(kt_sb[0][:, 0:128], kt_d[0, :, 0:128])
            nc.scalar.dma_start(qt_sb[0][:, 0:512], qt_d[0, :, 0:512])
            nc.sync.dma_start(qt_sb[0][:, 512:1024], qt_d[0, :, 512:1024])
            nc.gpsimd.dma_start(kt_sb[0][:, 128:512], kt_d[0, :, 128:512])
            nc.scalar.dma_start(va_sb[:, 0:1], va_d[:, 0:1])
            nc.sync.dma_start(va_sb[:, 1:4], va_d[:, 1:4])
            nc.sync.dma_start(kt_sb[0][:, 512:2048], kt_d[0, :, 512:2048])
            nc.sync.dma_start(va_sb[:, 4:10], va_d[:, 4:10])
            nc.sync.dma_start(va_sb[:, 10:16], va_d[:, 10:16])
            for o in range(1, 4):
                nc.sync.dma_start(kt_sb[o], kt_d[o])
                nc.sync.dma_start(qt_sb[o], qt_d[o])
            nc.sync.dma_start(idn_sb, idn_d[:])
            for o in range(4):
                nc.sync.dma_start(wt_sb[:, o : o + 1], wt_d[o : o + 1])

            # per-pair psum/staging tiles, captured per pair generation so
            # deferred work (normalize/transpose) reads the right tiles
            pair_state = {}

            def emit_qk_exp(hp, jt, ih, u):
                # One PSUM tile per head per unit, each with exactly ONE
                # reader engine: the tile framework serializes cross-engine
                # READERS of a PSUM tile (reader chaining), so sharing one S
                # tile between ACT and DVE would run them back-to-back.
                buf = u % 2
                s0 = ps_s.tile([128, 512], f32, tag=f"sa{buf}", name="s0")
                s1 = ps_s.tile([128, 512], f32, tag=f"sb{u % 3}", name="s1")
                for h01, st in ((0, s0), (1, s1)):
                    nc.tensor.matmul(
                        st,
                        lhsT=kt_sb[hp][64 * h01 : 64 * h01 + 64, jt * 128 : (jt + 1) * 128],
                        rhs=qt_sb[hp][64 * h01 : 64 * h01 + 64, ih * 512 : (ih + 1) * 512],
                        start=True,
                        stop=True,
                    )
                # GPSIMD cannot read PSUM on real HW, so only ACT and DVE
                # can consume S. DVE does head1's bit-trick exp on most
                # units; every 9th unit ACT takes head1 too (native exp) to
                # keep the DVE total under the PE-work bound.
                pa = ptp.tile([128, 512], bf, tag=f"pa{buf}", name="pa")
                nc.scalar.activation(pa, s0, Exp, scale=0.125)
                px = ptp.tile([128, 512], bf, tag=f"px{buf}", name="px")
                if u % 17 == 5:
                    # ACT absorbs head1 too on this cadence to balance DVE
                    nc.scalar.activation(px, s1, Exp, scale=0.125)
                else:
                    nc.vector.tensor_scalar(
                        px[:].bitcast(i16), s1, C1, C2, mult, add
                    )
                return (pa, px)

            def emit_pv(hp, jt, ih, p):
                o_tiles, sum_ps = pair_state[hp]["o"], pair_state[hp]["sum"]
                pa, px = p
                slices = [
                    [pa[:, 0:128], pa[:, 128:256], pa[:, 256:384], pa[:, 384:512]],
                    [px[:, 0:128], px[:, 128:256], px[:, 256:384], px[:, 384:512]],
                ]
                # PSUM accumulation groups are per 2KB zero-region (bank):
                # exactly one start (first touch zero-marks the whole bank)
                # and one stop (last touch) per o-bank / sum-bank per pair.
                # Last pair: slices 0-3 form their own accumulation group
                # that closes at (jt15, ih0), so normalize/transpose/proj of
                # the first i-half overlaps the final i-half's compute. The
                # ih1 writes keep accumulating with the group check skipped
                # (their bytes still carry the pending-zero marks from the
                # group-A start, so values stay exact).
                last = hp == 3
                for h01 in range(2):
                    for sl in range(4):
                        gsl = ih * 4 + sl
                        lhsT = slices[h01][sl]
                        o_stop = (
                            (jt == JT - 1 and gsl in (3, 7))
                            if last
                            else (jt == JT - 1 and gsl == 7)
                        )
                        nc.tensor.matmul(
                            o_tiles[h01][:, gsl, :],
                            lhsT=lhsT,
                            rhs=va_sb[:, jt, 2 * hp + h01, :],
                            start=(jt == 0 and gsl == 0),
                            stop=o_stop,
                            skip_group_check=(last and ih == 1),
                        )
                        s_stop = (
                            (jt == JT - 1 and gsl in (3, 7) and h01 == 1)
                            if last
                            else (jt == JT - 1 and gsl == 7 and h01 == 1)
                        )
                        nc.tensor.matmul(
                            sum_ps[:, h01 * 8 + gsl : h01 * 8 + gsl + 1],
                            lhsT=lhsT,
                            rhs=ones_c,
                            start=(jt == 0 and gsl == 0 and h01 == 0),
                            stop=s_stop,
                            skip_group_check=(last and ih == 1),
                        )

            def emit_norm(hp):
                # softmax normalization for the whole pair (PSUM accumulation
                # groups close at the pair's last PV, and mid-group reads are
                # not allowed): recip of sumexp, broadcast-multiply, bf16 out
                # staged for transpose
                st = pair_state[hp]
                o_tiles, sum_ps = st["o"], st["sum"]
                rc, on = st["rc"], st["on"]
                nc.vector.reciprocal(rc, sum_ps[:, 0:16])
                for h01 in range(2):
                    nc.vector.tensor_tensor(
                        on[:, :, 64 * h01 : 64 * h01 + 64],
                        o_tiles[h01],
                        rc[:, h01 * 8 : h01 * 8 + 8].unsqueeze(2).broadcast_to(
                            [128, 8, HD]
                        ),
                        mult,
                    )
                if debug and hp == 0:
                    dbg_o = work.tile([128, 8, HD], f32, tag="dbg_o", name="dbg_o")
                    nc.vector.tensor_copy(dbg_o, o_tiles[0])
                    nc.sync.dma_start(dbg["dbg_o"][:], dbg_o)

            def emit_norm_half(ih):
                st = pair_state[3]
                o_tiles, sum_ps = st["o"], st["sum"]
                rc, on = st["rc"], st["on"]
                a = ih * 4
                for h01 in range(2):
                    nc.vector.reciprocal(
                        rc[:, h01 * 8 + a : h01 * 8 + a + 4],
                        sum_ps[:, h01 * 8 + a : h01 * 8 + a + 4],
                    )
                    nc.vector.tensor_tensor(
                        on[:, a : a + 4, 64 * h01 : 64 * h01 + 64],
                        o_tiles[h01][:, a : a + 4, :],
                        rc[:, h01 * 8 + a : h01 * 8 + a + 4]
                        .unsqueeze(2)
                        .broadcast_to([128, 4, HD]),
                        mult,
                    )

            proj_state = {}

            def emit_proj_partial(c, tag):
                # pairs 0-2 of chunk c (group left open; finished later once
                # ot3 lands). Banks: the sa/sb psum tags free up as the last
                # units retire, giving four chunks in flight.
                csl = slice(c * 128, (c + 1) * 128)
                ps_f = ps_s.tile([128, D], f32, tag=tag, name="ps_f")
                proj_state[c] = ps_f
                for o in range(3):
                    nc.tensor.matmul(
                        ps_f,
                        lhsT=ot_sb[o][:, csl],
                        rhs=wt_sb[:, o, :],
                        start=(o == 0),
                        stop=False,
                    )

            def emit_proj_finish(c):
                csl = slice(c * 128, (c + 1) * 128)
                ps_f = proj_state.pop(c)
                nc.tensor.matmul(
                    ps_f, lhsT=ot_sb[3][:, csl], rhs=wt_sb[:, 3, :],
                    start=False, stop=True,
                )
                f_sb = ptp.tile([128, D], bf, tag="fin", name="f_sb", bufs=6)
                if c % 2 == 0:
                    nc.vector.tensor_copy(f_sb, ps_f)
                else:
                    nc.scalar.copy(f_sb, ps_f)
                # alternate output-DMA queues: the SP SEQ costs ~0.65us per
                # issue and head-of-line blocks on the copy, so the idle
                # Pool (SWDGE) queue takes every other chunk
                if c % 2 == 0:
                    nc.sync.dma_start(out_d[csl, :], f_sb)
                else:
                    nc.gpsimd.dma_start(out_d[csl, :], f_sb)

            def emit_proj(c, tag):
                emit_proj_partial(c, tag)
                emit_proj_finish(c)

            def emit_tail_half(ih):
                # transposes of the half's 4 slices, copy to SBUF, then the
                # matching projection chunks
                st = pair_state[3]
                on = st["on"]
                if ih == 0:
                    tpl = ps_s.tile([128, 512], f32, tag="sb0", name="tp_last")
                    st["tpl"] = tpl
                    tgt = tpl[:].bitcast(bf)
                else:
                    tgt = st["sum"][:].bitcast(bf)
                for sl in range(ih * 4, ih * 4 + 4):
                    nc.tensor.matmul(
                        tgt[:, (sl % 4) * 128 : (sl % 4) * 128 + 128]
                        if ih == 0
                        else tgt[:, sl * 128 : sl * 128 + 128],
                        lhsT=on[:, sl, :],
                        rhs=idn_sb,
                        is_transpose=True,
                        start=(sl % 4 == 0),
                        stop=(sl % 4 == 3),
                    )
                src_cols = slice(0, 512) if ih == 0 else slice(512, 1024)
                if ih == 0:
                    nc.vector.tensor_copy(
                        ot_sb[3][:, 0:512], tgt[:, src_cols]
                    )
                    for c, tag in ((0, "sa0"), (1, "sa1"), (2, "sb2"), (3, "sa0")):
                        emit_proj(c, tag)
                    for c, tag in ((4, "sb1"), (5, "sa1"), (6, "sb2"), (7, "sa0")):
                        emit_proj_partial(c, tag)
                else:
                    nc.scalar.copy(ot_sb[3][:, 512:1024], tgt[:, src_cols])
                    for c in range(4, 8):
                        emit_proj_finish(c)

            def emit_transpose(hp):
                # The sum bank is dead after the normalize reads it, so the
                # pair's 8 transposes reuse it (one accumulation group, each
                # byte written exactly once), then a DMA moves O^T to SBUF —
                # no engine cycles spent on the copy.
                st = pair_state[hp]
                on = st["on"]
                tp = st["sum"][:].bitcast(bf)
                for sl in range(8):
                    nc.tensor.matmul(
                        tp[:, sl * 128 : (sl + 1) * 128],
                        lhsT=on[:, sl, :],
                        rhs=idn_sb,
                        is_transpose=True,
                        start=(sl == 0),
                        stop=(sl == 7),
                    )
                if hp == 3:
                    nc.scalar.copy(ot_sb[hp], tp)
                else:
                    nc.vector.tensor_copy(ot_sb[hp], tp)

            def alloc_pair(hp):
                pair_state[hp] = {
                    "o": [
                        ps_o.tile([128, 8, HD], f32, tag=f"o{h01}", name=f"o{h01}")
                        for h01 in range(2)
                    ],
                    "sum": ps_o.tile([128, 512], f32, tag="sum", name="sum"),
                    "rc": work.tile([128, 16], f32, tag="rc", name="rc"),
                    "on": work.tile([128, 8, 128], bf, tag="on", name="on"),
                }
                if hp - 2 in pair_state:
                    del pair_state[hp - 2]

            # Software-pipelined emission, PV lagging QK/exp by LAG units so
            # the PE never waits on the exp engines in steady state. Pair
            # boundary work is interleaved: norm right after the pair's last
            # PV, transposes one iteration later (so the PE meets them after
            # the DVE normalize has finished), all before the next pair
            # reuses the same psum tags.
            LAG = 5
            units = [(hp, jt, ih) for hp in range(4) for jt in range(JT) for ih in range(NU)]
            transposes = []  # (due_iter, hp)
            tails = []  # (due_iter, ih) for the last pair

            def emit_iter(u):
                # PV of u-LAG first: its inputs are ready, so the in-order
                # PE SEQ does useful work while QK(u) waits out the s-buffer
                # WAR on the exp engines of u-2.
                pu = u - LAG
                if 0 <= pu < len(units):
                    php, pjt, pih = units[pu]
                    if pjt == 0 and pih == 0:
                        alloc_pair(php)
                    emit_pv(php, pjt, pih, pending_p[pu])
                    pending_p[pu] = None
                # norm before this unit's exp: it then sits ahead of the
                # DVE's next bit-exp in the queue, starting the pair-close
                # chain one exp earlier (sb's 3-deep buffering absorbs the
                # delayed exp)
                if 0 <= pu < len(units):
                    php, pjt, pih = units[pu]
                    if pjt == JT - 1:
                        if php == 3:
                            emit_norm_half(pih)
                            tails.append((u + 1, pih))
                        elif pih == 1:
                            emit_norm(php)
                            transposes.append((u, php))
                if u < len(units):
                    hp, jt, ih = units[u]
                    emit_qk_exp(hp, jt, ih, u)
                while transposes and transposes[0][0] <= u:
                    _, thp = transposes.pop(0)
                    emit_transpose(thp)
                while tails and tails[0][0] <= u:
                    _, tih = tails.pop(0)
                    emit_tail_half(tih)

            pending_p = {}
            orig_qk = emit_qk_exp

            def emit_qk_exp_wrap(hp, jt, ih, u):
                pending_p[u] = orig_qk(hp, jt, ih, u)

            emit_qk_exp = emit_qk_exp_wrap
            for u in range(len(units) + LAG):
                emit_iter(u)
            while transposes:
                _, thp = transposes.pop(0)
                emit_transpose(thp)
            while tails:
                _, tih = tails.pop(0)
                emit_tail_half(tih)


    nc.finalize()
    return nc


def _host_prep(q, k, v, W_out, b_out):
    """Shard + lay out inputs per core (pure layout: transpose/pack/bf16)."""
    import ml_dtypes

    bf16 = ml_dtypes.bfloat16
    q = np.asarray(q, dtype=np.float32)
    k = np.asarray(k, dtype=np.float32)
    v = np.asarray(v, dtype=np.float32)
    W_out = np.asarray(W_out, dtype=np.float32)
    b_out = np.asarray(b_out, dtype=np.float32)

    qT = np.ascontiguousarray(q.transpose(0, 2, 1)).astype(bf16)  # [B, D, N]
    kT = np.ascontiguousarray(k.transpose(0, 2, 1)).astype(bf16)  # [B, D, M]
    # va[p, jt, h, hd] = v[b, jt*128 + p, h*64 + hd]
    va = np.ascontiguousarray(
        v.reshape(B, JT, 128, H, HD).transpose(0, 2, 1, 3, 4)
    ).astype(bf16)
    wt = np.ascontiguousarray(W_out.T.reshape(4, 128, D)).astype(bf16)
    idn = np.eye(128, dtype=np.float32).astype(bf16)

    in_maps = []
    for c in range(NCORES):
        b_, ihalf = divmod(c, 2)
        in_maps.append(
            {
                "qt": np.ascontiguousarray(
                    qT[b_, :, ihalf * NLOC : (ihalf + 1) * NLOC].reshape(4, 128, NLOC)
                ),
                "kt": np.ascontiguousarray(kT[b_].reshape(4, 128, M)),
                "va": va[b_],
                "wt": wt,
                "idn": idn,
            }
        )
    return in_maps


def kernel(q, k, v, W_out, b_out):
    from concourse.bass_utils import run_bass_kernel_spmd

    nc = _build_bass()
    in_maps = _host_prep(q, k, v, W_out, b_out)
    res = run_bass_kernel_spmd(nc, in_maps, core_ids=list(range(NCORES)))
    out = np.empty((B, N, D), dtype=np.float32)
    for c, r_ in enumerate(res.results):
        b_, ihalf = divmod(c, 2)
        out[b_, ihalf * NLOC : (ihalf + 1) * NLOC, :] = r_["out"].astype(
            np.float32
        )
    b_vec = np.asarray(b_out, dtype=np.float32)
    if b_vec.any():
        out += b_vec[None, None, :]
    return out
